# revision 35
# baseline (speedup 1.0000x reference)
"""ExLlama transformer layer (GPTQ int4) on 8 TRN2 NeuronCores, tensor-parallel.

Self-contained: hardcodes shapes from the problem spec.
  B=1, S=2048, HID=4096, INTER=11008, HEADS=32, HD=128, GS=128.

v3: host-side dequantization to bf16 with ln1/ln2 folded into the qkv /
gate/up weights; the rms normalization factor (rstd) is applied POST-matmul
(folded into the rope cos/sin tables per column and the v/x scaling), so
qkv matmuls depend only on the raw h stash.  h/ar strips DMA directly into
the stash layout (no copy).  o partials carry +h/8 so the AllReduce output
is h2.  MLP runs in 2 half-sequence passes; pools/tags are reused across
phases so SBUF addresses hand over without phase-boundary stalls.  The
final ReduceScatter runs per 512-column chunk, overlapped under compute.
DMA issue is spread across the sync/scalar/gpsimd queues.
"""
import sys

sys.path.insert(0, "/opt/trn_rl_repo")

import numpy as np

S = 2048
HID = 4096
HD = 128
GS = 128
INTER = 11008
NCORES = 8
IPC = 1408                      # padded inter features per core
IPAD = IPC * NCORES             # 11264
NKT = HID // 128                # 32 k-tiles over HID
NIT = IPC // 128                # 11 k-tiles over per-core inter
OPC = HID // NCORES             # 512 out features per core (qkv), 4 heads
NHC = OPC // HD                 # 4 heads per core
NST = S // 128                  # 16 s-tiles
SH = S // 2                     # 1024, seq half
SCALE = 1.0 / float(np.sqrt(HD))
EPS = 1e-6
NEG = -30000.0

_BUILD_CACHE = {}


def _build():
    import concourse.bacc as bacc
    import concourse.mybir as mybir
    import concourse.tile as tile
    import ml_dtypes

    dt = mybir.dt
    F32, F16, BF16, I32 = dt.float32, dt.float16, dt.bfloat16, dt.int32
    Alu = mybir.AluOpType
    Act = mybir.ActivationFunctionType

    nc = bacc.Bacc("TRN2", target_bir_lowering=False, num_devices=NCORES)

    # ---------------- external I/O ----------------
    hT_d = nc.dram_tensor("hT", [HID, S], BF16, kind="ExternalInput")
    cosT_d = nc.dram_tensor("cosT", [HD, S], F16, kind="ExternalInput")
    sinfT_d = nc.dram_tensor("sinfT", [HD, S], F16, kind="ExternalInput")
    # bf16 weight slabs pre-arranged so each slab DMA is a plain 2D copy:
    # row block = [128 k-partitions, batch*nkt*128] per (batched) out-tile
    wqkv_d = nc.dram_tensor("wqkv", [12 * 128, NKT * 128], BF16,
                            kind="ExternalInput")
    wo_d = nc.dram_tensor("wo", [8 * 128, 4 * NHC * 128], BF16,
                          kind="ExternalInput")
    wgu_d = nc.dram_tensor("wgu", [2 * NIT * 128, NKT * 128], BF16,
                           kind="ExternalInput")
    wdn_d = nc.dram_tensor("wdn", [16 * 128, 2 * NIT * 128], BF16,
                           kind="ExternalInput")

    out_d = nc.dram_tensor("out", [OPC, S], F16, kind="ExternalOutput")

    # ---------------- inline constants ----------------
    p = np.arange(128)
    id16_c = nc.inline_tensor(np.eye(128, dtype=np.float16), name="id16")
    ones16_c = nc.inline_tensor(np.ones((128, 1), np.float16), name="ones16")
    # wide causal mask, S^T layout: mkw[k, j] with qq = j-384: 0 if qq >= k
    # else NEG.  Slice [384-128*dd : 896-128*dd] = mask for diag offset dd.
    j = np.arange(896)
    mkw = np.where((j[None, :] - 384) >= p[:, None], 0.0, NEG)
    mask_c = nc.inline_tensor(mkw.astype(ml_dtypes.bfloat16), name="maskc")

    with tile.TileContext(nc) as tc:
        ctx_pools = []

        def open_pool(**kw):
            cm = tc.tile_pool(**kw)
            pool = cm.__enter__()
            ctx_pools.append((cm, kw["name"]))
            return pool

        def close_pool(pool_name):
            for i, (cm, nm) in enumerate(ctx_pools):
                if nm == pool_name:
                    cm.__exit__(None, None, None)
                    ctx_pools.pop(i)
                    return

        cp = open_pool(name="const", bufs=1)
        dp = open_pool(name="dram", bufs=1, space="DRAM")
        w4 = open_pool(name="wk4", bufs=2)    # f32t [128,512]
        w6 = open_pool(name="wk6", bufs=3)    # f16t [128,512]
        w8 = open_pool(name="wk8", bufs=2)    # wide f16 [128,1024]
        we = open_pool(name="wke", bufs=2)    # ET [128,512] bf16
        w3 = open_pool(name="wk3", bufs=1)    # rows [1,*]
        w2 = open_pool(name="wk2", bufs=2)    # onat, rz
        rp = open_pool(name="rope", bufs=1)   # per-half rope consts (wide)
        pp = open_pool(name="ps", bufs=3, space="PSUM")
        prs = open_pool(name="psr", bufs=1, space="PSUM")

        # ---- persistent consts in SBUF ----
        id16 = cp.tile([128, 128], F16, tag="id16")
        nc.sync.dma_start(out=id16[:], in_=id16_c[:])
        ones16 = cp.tile([128, 1], F16, tag="ones16")
        nc.sync.dma_start(out=ones16[:], in_=ones16_c[:])
        maskt = cp.tile([128, 896], BF16, tag="maskt")
        nc.sync.dma_start(out=maskt[:], in_=mask_c[:])


        # ---- DRAM scratch ----
        part1_d = [dp.tile([HID, SH], F16, tag=f"part1_{i}", name=f"part1_{i}")
                   for i in range(2)]
        ar1_d = [dp.tile([HID, SH], F16, tag=f"ar1_{i}", name=f"ar1_{i}",
                         addr_space="Shared")
                 for i in range(2)]
        part2_d = [dp.tile([HID, 512], F16, tag=f"part2_{i}",
                           name=f"part2_{i}") for i in range(4)]
        rs2_d = [dp.tile([OPC, 512], F16, tag=f"rs2_{i}", name=f"rs2_{i}")
                 for i in range(4)]

        def f32t():
            return w4.tile([128, 512], F32, tag="f32t", name="f32t")

        def f16t(dtp=F16):
            return w6.tile([128, 512], dtp, tag="f16t", name="f16t")

        # =============== SBUF pools reused across phases ===============
        xp = open_pool(name="xph", bufs=1)    # x1T / x2Tc (same tag slot)
        qk2 = open_pool(name="qk2", bufs=1)   # qTa
        qk1 = open_pool(name="qk1", bufs=1)   # kTa, Vn (later yTc)
        op_ = open_pool(name="oTp", bufs=2)   # oTh
        wqp = open_pool(name="wq", bufs=3)    # [128,4096] slabs: qkv, gate/up
        wop = open_pool(name="wo", bufs=2)    # o (2048) / down (2816) slabs
        hop = open_pool(name="ho", bufs=2)    # h / h2 row slices
        pro = open_pool(name="pso", bufs=3, space="PSUM")
        ptr = open_pool(name="pst", bufs=1, space="PSUM")

        kTa = qk1.tile([128, NHC * S], F16, tag="kTa")
        Vn = qk1.tile([128, NHC * NST * 132], BF16, tag="Vn")

        def rms_pass(src_d, c0, stash_tag, stash_dt, with_rope):
            """DMA 1024 cols of src into stash layout, ssq -> rstd; build
            per-half rope consts (rstd folded).  Returns (stash, consts)."""
            xT = xp.tile([128, NKT * SH], stash_dt, tag="x1T", name="x1T")
            ssq = prs.tile([64, 512], F32, tag="rs")
            for st in range(8):
                nc.gpsimd.dma_start(
                    out=xT[:, st * 4 * SH:(st * 4 + 4) * SH]
                    .rearrange("p (kt s) -> p kt s", s=SH),
                    in_=src_d[st * 512:(st + 1) * 512, c0:c0 + SH]
                    .rearrange("(kt p) s -> p kt s", p=128))
                for k2 in range(4):
                    kt = st * 4 + k2
                    xsl = xT[:, kt * SH: kt * SH + SH]
                    sq = w8.tile([128, 1024], F16, tag="w8", name="w8")
                    nc.vector.tensor_tensor(out=sq[:], in0=xsl, in1=xsl,
                                            op=Alu.mult)
                    nc.tensor.matmul(ssq[0:1, :], ones16[:], sq[:, 0:512],
                                     start=(kt == 0), stop=(kt == NKT - 1),
                                     skip_group_check=True)
                    nc.tensor.matmul(ssq[32:33, :], ones16[:], sq[:, 512:1024],
                                     start=(kt == 0), stop=(kt == NKT - 1),
                                     skip_group_check=True)
            srw = w3.tile([1, 1024], F16, tag="srw", name="srw")
            for ch2 in range(2):
                trow = w3.tile([1, 512], F32, tag="rows")
                nc.vector.tensor_scalar(out=trow[:],
                                        in0=ssq[32 * ch2:32 * ch2 + 1, :],
                                        scalar1=1.0 / HID, scalar2=EPS,
                                        op0=Alu.mult, op1=Alu.add)
                nc.vector.reciprocal(trow[:], trow[:])
                nc.scalar.activation(srw[0:1, ch2 * 512:ch2 * 512 + 512],
                                     trow[:], Act.Sqrt)
            rstdB = rp.tile([128, 1024], F16, tag="rstdB", name="rstdB")
            nc.gpsimd.partition_broadcast(rstdB[:], srw[:])
            consts = None
            if with_rope:
                csl = rp.tile([128, 1024], F16, tag="csl", name="csl")
                nc.sync.dma_start(out=csl[:], in_=cosT_d[:, c0:c0 + SH])
                ssl = rp.tile([128, 1024], F16, tag="ssl", name="ssl")
                nc.sync.dma_start(out=ssl[:], in_=sinfT_d[:, c0:c0 + SH])
                cq = rp.tile([128, 1024], F16, tag="cq", name="cq")
                nc.vector.tensor_tensor(out=cq[:], in0=csl[:],
                                        in1=rstdB[:], op=Alu.mult)
                sq_ = rp.tile([128, 1024], F16, tag="sq_", name="sq_")
                nc.vector.tensor_tensor(out=sq_[:], in0=ssl[:],
                                        in1=rstdB[:], op=Alu.mult)
                ck = rp.tile([128, 1024], F16, tag="ck", name="ck")
                nc.vector.scalar_tensor_tensor(
                    out=ck[:], in0=csl[:], scalar=SCALE,
                    in1=rstdB[:], op0=Alu.mult, op1=Alu.mult)
                sk = rp.tile([128, 1024], F16, tag="sk", name="sk")
                nc.vector.scalar_tensor_tensor(
                    out=sk[:], in0=ssl[:], scalar=SCALE,
                    in1=rstdB[:], op0=Alu.mult, op1=Alu.mult)
                consts = (cq, sq_, ck, sk)
            return xT, rstdB, consts

        xr = rms_pass(hT_d, 0, "a", BF16, True)
        for sh in range(2):
            x1T, rstdB, (cq, sq_, ck, sk) = xr
            qTa = qk2.tile([128, NHC * SH], F16, tag="qTa")
            # ---- qkv for all heads, this half ----
            for ot in range(12):
                wsl = wqp.tile([128, NKT * 128], BF16, tag="wsl", name="wsl")
                nc.sync.dma_start(out=wsl[:],
                                  in_=wqkv_d[ot * 128:(ot + 1) * 128, :])
                mm = [pp.tile([128, 512], F32, tag="mm", name="mm")
                      for _ in range(2)]
                for kt in range(NKT):
                    for c2 in range(2):
                        nc.tensor.matmul(
                            mm[c2][:], wsl[:, kt * 128:(kt + 1) * 128],
                            x1T[:, kt * SH + c2 * 512: kt * SH + c2 * 512 + 512],
                            start=(kt == 0), stop=(kt == NKT - 1),
                            skip_group_check=True)
                h = ot % NHC
                which = ("q", "k", "v")[ot // NHC]
                for ch2 in range(2):
                    cc = ch2 * 512
                    gc0 = sh * SH + cc      # global col
                    if which in ("q", "k"):
                        qsb = f16t()
                        nc.scalar.activation(qsb[:], mm[ch2][:], Act.Copy)
                        qsh = f16t()
                        nc.scalar.dma_start(out=qsh[0:64, :],
                                            in_=qsb[64:128, :])
                        nc.scalar.dma_start(out=qsh[64:128, :],
                                            in_=qsb[0:64, :])
                        cR, sR = (cq, sq_) if which == "q" else (ck, sk)
                        t1 = f16t()
                        nc.vector.tensor_tensor(
                            out=t1[:], in0=qsb[:],
                            in1=cR[:, cc:cc + 512], op=Alu.mult)
                        t2 = f16t()
                        nc.vector.tensor_tensor(
                            out=t2[:], in0=qsh[:],
                            in1=sR[:, cc:cc + 512], op=Alu.mult)
                        if which == "q":
                            dsl = qTa[:, h * SH + cc: h * SH + cc + 512]
                        else:
                            dsl = kTa[:, h * S + gc0: h * S + gc0 + 512]
                        nc.vector.tensor_tensor(out=dsl, in0=t1[:],
                                                in1=t2[:], op=Alu.add)
                    else:
                        vt = f16t()
                        nc.vector.tensor_tensor(out=vt[:], in0=mm[ch2][:],
                                                in1=rstdB[:, cc:cc + 512],
                                                op=Alu.mult)
                        for st4 in range(4):
                            st = sh * 8 + ch2 * 4 + st4
                            vo = h * NST * 132 + st * 132
                            trp = ptr.tile([128, 128], F16, tag="tr")
                            nc.tensor.transpose(
                                trp[:], vt[:, st4 * 128:(st4 + 1) * 128],
                                id16[:])
                            nc.scalar.activation(
                                Vn[:, vo: vo + 128], trp[:], Act.Copy)
                            nc.vector.memset(Vn[:, vo + 128: vo + 129], 1.0)
            if sh == 0:
                xr = rms_pass(hT_d, SH, "a", BF16, True)
            # ---- attention, q-chunks of this half ----
            oTh = op_.tile([128, NHC * SH], F16, tag="oTh")
            for qs2 in range(2):
                qs = sh * 2 + qs2
                npairs = 4 * qs + 4
                for h in range(NHC):
                    oap = [pro.tile([128, 264], F32, tag="oa", name="oa")
                           for _ in range(2)]
                    for jj in range(npairs):
                        scp = pp.tile([128, 512], F32, tag="mm")
                        nc.tensor.matmul(
                            scp[:], kTa[:, h * S + jj * 128: h * S + (jj + 1) * 128],
                            qTa[:, h * SH + qs2 * 512: h * SH + qs2 * 512 + 512],
                            start=True, stop=True)
                        ET = we.tile([128, 512], BF16, tag="ET",
                                     name="ET")
                        if jj >= 4 * qs:
                            dd = jj - 4 * qs
                            ms = f32t()
                            nc.vector.tensor_tensor(
                                out=ms[:], in0=scp[:],
                                in1=maskt[:, 384 - 128 * dd: 896 - 128 * dd],
                                op=Alu.add)
                            nc.scalar.activation(ET[:], ms[:], Act.Exp)
                        else:
                            nc.scalar.activation(ET[:], scp[:], Act.Exp)
                        for qt in range(4):
                            vo = h * NST * 132 + jj * 132
                            qo = (qt % 2) * 132
                            nc.tensor.matmul(
                                oap[qt // 2][:, qo: qo + 129],
                                ET[:, qt * 128:(qt + 1) * 128],
                                Vn[:, vo: vo + 129],
                                start=(jj == 0 and qt % 2 == 0),
                                stop=(jj == npairs - 1),
                                skip_group_check=True)
                    for qt in range(4):
                        oa = oap[qt // 2][:, (qt % 2) * 132:(qt % 2) * 132 + 132]
                        rz = w2.tile([128, 1], F32, tag="rz")
                        nc.vector.reciprocal(rz[:], oa[:, 128:129])
                        onat = w2.tile([128, 128], F16, tag="onat")
                        nc.vector.tensor_scalar(out=onat[:],
                                                in0=oa[:, 0:128],
                                                scalar1=rz[:], scalar2=None,
                                                op0=Alu.mult)
                        trp = ptr.tile([128, 128], F16, tag="tr")
                        nc.tensor.transpose(trp[:], onat[:], id16[:])
                        so = h * SH + (qs2 * 4 + qt) * 128
                        nc.scalar.activation(oTh[:, so: so + 128], trp[:],
                                             Act.Copy)
            # ---- o-projection partials (+h/8) for this half -> AllReduce ----
            for ot4 in range(0, NKT, 4):
                wsl = wop.tile([128, 4 * NHC * 128], BF16, tag="wos",
                               name="wos")
                nc.sync.dma_start(
                    out=wsl[:],
                    in_=wo_d[(ot4 // 4) * 128:(ot4 // 4 + 1) * 128, :])
                for o4 in range(4):
                    ot = ot4 + o4
                    hsl = hop.tile([128, 1024], BF16, tag="hsl", name="hsl")
                    nc.gpsimd.dma_start(
                        out=hsl[:],
                        in_=hT_d[ot * 128:(ot + 1) * 128,
                                 sh * SH:(sh + 1) * SH])
                    mm = [pp.tile([128, 512], F32, tag="mm", name="mm")
                          for _ in range(2)]
                    for kt in range(NHC):
                        for c2 in range(2):
                            nc.tensor.matmul(
                                mm[c2][:],
                                wsl[:, (o4 * NHC + kt) * 128:
                                    (o4 * NHC + kt + 1) * 128],
                                oTh[:, kt * SH + c2 * 512:
                                    kt * SH + c2 * 512 + 512],
                                start=(kt == 0), stop=(kt == NHC - 1),
                                skip_group_check=True)
                    pt = w8.tile([128, 1024], F16, tag="w8", name="w8")
                    for c2 in range(2):
                        nc.vector.scalar_tensor_tensor(
                            out=pt[:, c2 * 512:c2 * 512 + 512],
                            in0=hsl[:, c2 * 512:c2 * 512 + 512],
                            scalar=1.0 / NCORES, in1=mm[c2][:],
                            op0=Alu.mult, op1=Alu.add)
                    nc.scalar.dma_start(
                        out=part1_d[sh][ot * 128:(ot + 1) * 128, :],
                        in_=pt[:])
            nc.gpsimd.collective_compute(
                "AllReduce", Alu.add,
                replica_groups=[list(range(NCORES))],
                ins=[part1_d[sh][:].opt()], outs=[ar1_d[sh][:].opt()])
        close_pool("pst")
        close_pool("pso")

        # ====================== MLP super-phase =============================
        # Two passes over seq-halves; ar1[p] (= h2) is the input.
        pgu = open_pool(name="psgu", bufs=4, space="PSUM")

        for ps in range(2):
            arck = ar1_d[ps]
            # ---- rms2 over ar (=h2); x2 = ar * rstd (ln2 folded in W) ----
            x2Tc, rstd2B, _ = rms_pass(arck, 0, "b", F16, False)
            for kt in range(NKT):
                xsl = x2Tc[:, kt * SH: kt * SH + SH]
                nc.vector.tensor_tensor(out=xsl, in0=xsl, in1=rstd2B[:],
                                        op=Alu.mult)
            yTc = qk1.tile([128, NIT * SH], F16, tag="Vn", name="yTc")
            # ---- gate/up -> yTc ----
            for it in range(NIT):
                wg = wqp.tile([128, NKT * 128], BF16, tag="wsl", name="wsl")
                nc.sync.dma_start(out=wg[:],
                                  in_=wgu_d[it * 128:(it + 1) * 128, :])
                wu = wqp.tile([128, NKT * 128], BF16, tag="wsl", name="wsl")
                nc.sync.dma_start(
                    out=wu[:],
                    in_=wgu_d[(NIT + it) * 128:(NIT + it + 1) * 128, :])
                gp = [pgu.tile([128, 512], F32, tag="gup", name="gup")
                      for _ in range(2)]
                up = [pgu.tile([128, 512], F32, tag="gup", name="gup")
                      for _ in range(2)]
                for kt in range(NKT):
                    for c2 in range(2):
                        nc.tensor.matmul(
                            gp[c2][:], wg[:, kt * 128:(kt + 1) * 128],
                            x2Tc[:, kt * SH + c2 * 512:
                                 kt * SH + c2 * 512 + 512],
                            start=(kt == 0), stop=(kt == NKT - 1),
                            skip_group_check=True)
                for kt in range(NKT):
                    for c2 in range(2):
                        nc.tensor.matmul(
                            up[c2][:], wu[:, kt * 128:(kt + 1) * 128],
                            x2Tc[:, kt * SH + c2 * 512:
                                 kt * SH + c2 * 512 + 512],
                            start=(kt == 0), stop=(kt == NKT - 1),
                            skip_group_check=True)
                for c2 in range(2):
                    sg = f16t()
                    nc.scalar.activation(sg[:], gp[c2][:], Act.Silu)
                    nc.vector.tensor_tensor(
                        out=yTc[:, it * SH + c2 * 512:
                                it * SH + c2 * 512 + 512],
                        in0=sg[:], in1=up[c2][:], op=Alu.mult)
            # ---- down partials (+h2/8), per 512-chunk -> ReduceScatter ----
            for ch2 in range(2):
                gch = ps * 2 + ch2          # global 512-col chunk
                cc = ch2 * 512
                for ot2 in range(0, NKT, 2):
                    wsl = wop.tile([128, 2 * NIT * 128], BF16, tag="wos",
                                   name="wos")
                    nc.sync.dma_start(
                        out=wsl[:],
                        in_=wdn_d[(ot2 // 2) * 128:(ot2 // 2 + 1) * 128, :])
                    hsl = hop.tile([128, 1024], F16, tag="hsl", name="hsl")
                    nc.gpsimd.dma_start(
                        out=hsl[:].rearrange("p (o s) -> p o s", s=512),
                        in_=arck[ot2 * 128:(ot2 + 2) * 128, cc:cc + 512]
                        .rearrange("(o p) s -> p o s", p=128))
                    for o2 in range(2):
                        ot = ot2 + o2
                        mm = pp.tile([128, 512], F32, tag="mm")
                        for kt in range(NIT):
                            nc.tensor.matmul(
                                mm[:],
                                wsl[:, (o2 * NIT + kt) * 128:
                                    (o2 * NIT + kt + 1) * 128],
                                yTc[:, kt * SH + cc: kt * SH + cc + 512],
                                start=(kt == 0), stop=(kt == NIT - 1))
                        pt = f16t()
                        nc.vector.scalar_tensor_tensor(
                            out=pt[:], in0=hsl[:, o2 * 512:o2 * 512 + 512],
                            scalar=1.0 / NCORES, in1=mm[:],
                            op0=Alu.mult, op1=Alu.add)
                        nc.scalar.dma_start(
                            out=part2_d[gch][ot * 128:(ot + 1) * 128, :],
                            in_=pt[:])
                nc.gpsimd.collective_compute(
                    "ReduceScatter", Alu.add,
                    replica_groups=[list(range(NCORES))],
                    ins=[part2_d[gch][:].opt()], outs=[rs2_d[gch][:].opt()])
                nc.sync.dma_start(out=out_d[:, gch * 512:(gch + 1) * 512],
                                  in_=rs2_d[gch][:])
        close_pool("psgu")

        for cm, nm in reversed(ctx_pools):
            cm.__exit__(None, None, None)
        ctx_pools.clear()

    nc.compile()
    return nc


def _dequant_np(qw, qz, sc):
    """GPTQ int4 -> float32 weight matrix [in_f, out_f] (numpy)."""
    in_f, out_f = qw.shape[0] * 8, qw.shape[1]
    groups = sc.shape[0]
    sh = (np.arange(8, dtype=np.uint32) * 4)
    w = ((qw.view(np.uint32)[:, None, :] >> sh[None, :, None]) & 15)
    w = w.reshape(groups, in_f // groups, out_f).astype(np.float32)
    z = ((qz.view(np.uint32)[:, :, None] >> sh[None, None, :]) & 15)
    z = z.reshape(groups, out_f).astype(np.float32)
    return ((w - (z + 1.0)[:, None, :]) * sc[:, None, :]).reshape(in_f, out_f)


def _slab(w, nkt, batch=1):
    """[in_f, out_f] f32 -> slabs [(notile//batch)*128, batch*nkt*128] bf16.

    Row block b holds, for each k-partition p, the [ot-in-batch][kt][o]
    weights — so one plain 2D DMA delivers a ready lhsT slab."""
    import ml_dtypes
    in_f, out_f = w.shape
    assert in_f == nkt * 128
    notile = out_f // 128
    s = w.reshape(nkt, 128, notile, 128).transpose(2, 0, 1, 3)  # [ot,kt,p,o]
    s = s.reshape(notile // batch, batch, nkt, 128, 128)
    s = s.transpose(0, 3, 1, 2, 4)                        # [b, p, bot, kt, o]
    return np.ascontiguousarray(
        s.reshape(notile // batch * 128,
                  batch * nkt * 128)).astype(ml_dtypes.bfloat16)


def _host_prep(inputs):
    """Build the 8 per-core input maps from full inputs."""
    import ml_dtypes
    bf16 = ml_dtypes.bfloat16
    f16 = np.float16

    h = np.asarray(inputs["hidden_states"], np.float32)[0]     # [S, HID]
    hT = np.ascontiguousarray(h.T).astype(bf16)                # [HID, S]
    sin = np.asarray(inputs["sin"], np.float32)                # [S, HD]
    cos = np.asarray(inputs["cos"], np.float32)
    cosT = np.ascontiguousarray(cos.T).astype(f16)
    sinf = sin.T.copy()
    sinf[0:64, :] *= -1.0                                      # rot-half sign fold
    sinfT = np.ascontiguousarray(sinf).astype(f16)

    dq = {}
    for nm in ("q", "k", "v", "o", "gate", "up", "down"):
        dq[nm] = _dequant_np(np.asarray(inputs["qw_" + nm]),
                             np.asarray(inputs["qz_" + nm]),
                             np.asarray(inputs["sc_" + nm], np.float32))

    ln1 = np.asarray(inputs["ln1_w"], np.float32)
    ln2 = np.asarray(inputs["ln2_w"], np.float32)
    for nm in ("q", "k", "v"):
        dq[nm] = dq[nm] * ln1[:, None]
    dq["gate"] = dq["gate"] * ln2[:, None]
    dq["up"] = dq["up"] * ln2[:, None]

    def pad_cols(a, wdt):
        out = np.zeros((a.shape[0], wdt), a.dtype)
        out[:, :a.shape[1]] = a
        return out

    w_g = pad_cols(dq["gate"], IPAD)
    w_u = pad_cols(dq["up"], IPAD)
    w_dn = np.zeros((IPAD, HID), np.float32)
    w_dn[:INTER] = dq["down"]

    maps = []
    for c in range(NCORES):
        cs = slice(c * OPC, (c + 1) * OPC)
        isl = slice(c * IPC, (c + 1) * IPC)
        qkv_cat = np.concatenate(
            [dq["q"][:, cs], dq["k"][:, cs], dq["v"][:, cs]], axis=1)
        gu_cat = np.concatenate([w_g[:, isl], w_u[:, isl]], axis=1)
        m = {
            "hT": hT, "cosT": cosT, "sinfT": sinfT,
            "wqkv": _slab(qkv_cat, NKT),
            "wo": _slab(dq["o"][cs, :], NHC, batch=4),
            "wgu": _slab(gu_cat, NKT),
            "wdn": _slab(w_dn[isl, :], NIT, batch=2),
        }
        maps.append(m)
    return maps


def run(inputs, trace=False):
    from concourse.bass_utils import run_bass_kernel_spmd
    if "rel" not in _BUILD_CACHE:
        _BUILD_CACHE["rel"] = _build()
    nc = _BUILD_CACHE["rel"]
    maps = _host_prep(inputs)
    res = run_bass_kernel_spmd(nc, maps, core_ids=list(range(NCORES)),
                               trace=trace)
    outT = np.empty((HID, S), np.float32)
    for c in range(NCORES):
        outT[c * OPC:(c + 1) * OPC] = res.results[c]["out"]
    out = np.ascontiguousarray(outT.T)[None]
    return out, res


def kernel(**inputs):
    out, _ = run(inputs)
    return out


# revision 36
# speedup vs baseline: 1.0097x; 1.0097x over previous
"""ExLlama transformer layer (GPTQ int4) on 8 TRN2 NeuronCores, tensor-parallel.

Self-contained: hardcodes shapes from the problem spec.
  B=1, S=2048, HID=4096, INTER=11008, HEADS=32, HD=128, GS=128.

v3: host-side dequantization to bf16 with ln1/ln2 folded into the qkv /
gate/up weights; the rms normalization factor (rstd) is applied POST-matmul
(folded into the rope cos/sin tables per column and the v/x scaling), so
qkv matmuls depend only on the raw h stash.  h/ar strips DMA directly into
the stash layout (no copy).  o partials carry +h/8 so the AllReduce output
is h2.  MLP runs in 2 half-sequence passes; pools/tags are reused across
phases so SBUF addresses hand over without phase-boundary stalls.  The
final ReduceScatter runs per 512-column chunk, overlapped under compute.
DMA issue is spread across the sync/scalar/gpsimd queues.
"""
import sys

sys.path.insert(0, "/opt/trn_rl_repo")

import numpy as np

S = 2048
HID = 4096
HD = 128
GS = 128
INTER = 11008
NCORES = 8
IPC = 1408                      # padded inter features per core
IPAD = IPC * NCORES             # 11264
NKT = HID // 128                # 32 k-tiles over HID
NIT = IPC // 128                # 11 k-tiles over per-core inter
OPC = HID // NCORES             # 512 out features per core (qkv), 4 heads
NHC = OPC // HD                 # 4 heads per core
NST = S // 128                  # 16 s-tiles
SH = S // 2                     # 1024, seq half
SCALE = 1.0 / float(np.sqrt(HD))
EPS = 1e-6
NEG = -30000.0

_BUILD_CACHE = {}


def _build():
    import concourse.bacc as bacc
    import concourse.mybir as mybir
    import concourse.tile as tile
    import ml_dtypes

    dt = mybir.dt
    F32, F16, BF16, I32 = dt.float32, dt.float16, dt.bfloat16, dt.int32
    Alu = mybir.AluOpType
    Act = mybir.ActivationFunctionType

    nc = bacc.Bacc("TRN2", target_bir_lowering=False, num_devices=NCORES)

    # ---------------- external I/O ----------------
    hT_d = nc.dram_tensor("hT", [HID, S], BF16, kind="ExternalInput")
    cosT_d = nc.dram_tensor("cosT", [HD, S], F16, kind="ExternalInput")
    sinfT_d = nc.dram_tensor("sinfT", [HD, S], F16, kind="ExternalInput")
    # bf16 weight slabs pre-arranged so each slab DMA is a plain 2D copy:
    # row block = [128 k-partitions, batch*nkt*128] per (batched) out-tile
    wqkv_d = nc.dram_tensor("wqkv", [12 * 128, NKT * 128], BF16,
                            kind="ExternalInput")
    wo_d = nc.dram_tensor("wo", [8 * 128, 4 * NHC * 128], BF16,
                          kind="ExternalInput")
    wgu_d = nc.dram_tensor("wgu", [2 * NIT * 128, NKT * 128], BF16,
                           kind="ExternalInput")
    wdn_d = nc.dram_tensor("wdn", [16 * 128, 2 * NIT * 128], BF16,
                           kind="ExternalInput")

    out_d = nc.dram_tensor("out", [OPC, S], F16, kind="ExternalOutput")

    # ---------------- inline constants ----------------
    p = np.arange(128)
    id16_c = nc.inline_tensor(np.eye(128, dtype=np.float16), name="id16")
    ones16_c = nc.inline_tensor(np.ones((128, 1), np.float16), name="ones16")
    # wide causal mask, S^T layout: mkw[k, j] with qq = j-384: 0 if qq >= k
    # else NEG.  Slice [384-128*dd : 896-128*dd] = mask for diag offset dd.
    j = np.arange(896)
    mkw = np.where((j[None, :] - 384) >= p[:, None], 0.0, NEG)
    mask_c = nc.inline_tensor(mkw.astype(ml_dtypes.bfloat16), name="maskc")

    with tile.TileContext(nc) as tc:
        ctx_pools = []

        def open_pool(**kw):
            cm = tc.tile_pool(**kw)
            pool = cm.__enter__()
            ctx_pools.append((cm, kw["name"]))
            return pool

        def close_pool(pool_name):
            for i, (cm, nm) in enumerate(ctx_pools):
                if nm == pool_name:
                    cm.__exit__(None, None, None)
                    ctx_pools.pop(i)
                    return

        cp = open_pool(name="const", bufs=1)
        dp = open_pool(name="dram", bufs=1, space="DRAM")
        w4 = open_pool(name="wk4", bufs=2)    # f32t [128,512]
        w6 = open_pool(name="wk6", bufs=3)    # f16t [128,512]
        w8 = open_pool(name="wk8", bufs=2)    # wide f16 [128,1024]
        we = open_pool(name="wke", bufs=2)    # ET [128,512] bf16
        w3 = open_pool(name="wk3", bufs=1)    # rows [1,*]
        w2 = open_pool(name="wk2", bufs=2)    # onat, rz
        rp = open_pool(name="rope", bufs=1)   # per-half rope consts (wide)
        pp = open_pool(name="ps", bufs=3, space="PSUM")
        prs = open_pool(name="psr", bufs=1, space="PSUM")

        # ---- persistent consts in SBUF ----
        id16 = cp.tile([128, 128], F16, tag="id16")
        nc.sync.dma_start(out=id16[:], in_=id16_c[:])
        ones16 = cp.tile([128, 1], F16, tag="ones16")
        nc.sync.dma_start(out=ones16[:], in_=ones16_c[:])
        maskt = cp.tile([128, 896], BF16, tag="maskt")
        nc.sync.dma_start(out=maskt[:], in_=mask_c[:])


        # ---- DRAM scratch ----
        part1_d = [dp.tile([HID, SH], F16, tag=f"part1_{i}", name=f"part1_{i}")
                   for i in range(2)]
        ar1_d = [dp.tile([HID, SH], F16, tag=f"ar1_{i}", name=f"ar1_{i}",
                         addr_space="Shared")
                 for i in range(2)]
        part2_d = [dp.tile([HID, 512], F16, tag=f"part2_{i}",
                           name=f"part2_{i}") for i in range(4)]
        rs2_d = [dp.tile([OPC, 512], F16, tag=f"rs2_{i}", name=f"rs2_{i}")
                 for i in range(4)]

        def f32t():
            return w4.tile([128, 512], F32, tag="f32t", name="f32t")

        def f16t(dtp=F16):
            return w6.tile([128, 512], dtp, tag="f16t", name="f16t")

        # =============== SBUF pools reused across phases ===============
        xp = open_pool(name="xph", bufs=1)    # x1T / x2Tc (same tag slot)
        qk2 = open_pool(name="qk2", bufs=1)   # qTa
        qk1 = open_pool(name="qk1", bufs=1)   # kTa, Vn (later yTc)
        op_ = open_pool(name="oTp", bufs=2)   # oTh
        wqp = open_pool(name="wq", bufs=3)    # [128,4096] slabs: qkv, gate/up
        wop = open_pool(name="wo", bufs=2)    # o (2048) / down (2816) slabs
        hop = open_pool(name="ho", bufs=2)    # h / h2 row slices
        pro = open_pool(name="pso", bufs=3, space="PSUM")
        ptr = open_pool(name="pst", bufs=1, space="PSUM")

        kTa = qk1.tile([128, NHC * S], F16, tag="kTa")
        Vn = qk1.tile([128, NHC * NST * 132], BF16, tag="Vn")

        def rms_pass(src_d, c0, stash_tag, stash_dt, with_rope):
            """DMA 1024 cols of src into stash layout, ssq -> rstd; build
            per-half rope consts (rstd folded).  Returns (stash, consts)."""
            xT = xp.tile([128, NKT * SH], stash_dt, tag="x1T", name="x1T")
            ssq = prs.tile([64, 512], F32, tag="rs")
            for st in range(8):
                nc.gpsimd.dma_start(
                    out=xT[:, st * 4 * SH:(st * 4 + 4) * SH]
                    .rearrange("p (kt s) -> p kt s", s=SH),
                    in_=src_d[st * 512:(st + 1) * 512, c0:c0 + SH]
                    .rearrange("(kt p) s -> p kt s", p=128))
                for k2 in range(4):
                    kt = st * 4 + k2
                    xsl = xT[:, kt * SH: kt * SH + SH]
                    sq = w8.tile([128, 1024], F16, tag="w8", name="w8")
                    nc.vector.tensor_tensor(out=sq[:], in0=xsl, in1=xsl,
                                            op=Alu.mult)
                    nc.tensor.matmul(ssq[0:1, :], ones16[:], sq[:, 0:512],
                                     start=(kt == 0), stop=(kt == NKT - 1),
                                     skip_group_check=True)
                    nc.tensor.matmul(ssq[32:33, :], ones16[:], sq[:, 512:1024],
                                     start=(kt == 0), stop=(kt == NKT - 1),
                                     skip_group_check=True)
            srw = w3.tile([1, 1024], F16, tag="srw", name="srw")
            for ch2 in range(2):
                trow = w3.tile([1, 512], F32, tag="rows")
                nc.vector.tensor_scalar(out=trow[:],
                                        in0=ssq[32 * ch2:32 * ch2 + 1, :],
                                        scalar1=1.0 / HID, scalar2=EPS,
                                        op0=Alu.mult, op1=Alu.add)
                nc.vector.reciprocal(trow[:], trow[:])
                nc.scalar.activation(srw[0:1, ch2 * 512:ch2 * 512 + 512],
                                     trow[:], Act.Sqrt)
            rstdB = rp.tile([128, 1024], F16, tag="rstdB", name="rstdB")
            nc.gpsimd.partition_broadcast(rstdB[:], srw[:])
            consts = None
            if with_rope:
                csl = rp.tile([128, 1024], F16, tag="csl", name="csl")
                nc.sync.dma_start(out=csl[:], in_=cosT_d[:, c0:c0 + SH])
                ssl = rp.tile([128, 1024], F16, tag="ssl", name="ssl")
                nc.sync.dma_start(out=ssl[:], in_=sinfT_d[:, c0:c0 + SH])
                cq = rp.tile([128, 1024], F16, tag="cq", name="cq")
                nc.vector.tensor_tensor(out=cq[:], in0=csl[:],
                                        in1=rstdB[:], op=Alu.mult)
                sq_ = rp.tile([128, 1024], F16, tag="sq_", name="sq_")
                nc.vector.tensor_tensor(out=sq_[:], in0=ssl[:],
                                        in1=rstdB[:], op=Alu.mult)
                ck = rp.tile([128, 1024], F16, tag="ck", name="ck")
                nc.vector.scalar_tensor_tensor(
                    out=ck[:], in0=csl[:], scalar=SCALE,
                    in1=rstdB[:], op0=Alu.mult, op1=Alu.mult)
                sk = rp.tile([128, 1024], F16, tag="sk", name="sk")
                nc.vector.scalar_tensor_tensor(
                    out=sk[:], in0=ssl[:], scalar=SCALE,
                    in1=rstdB[:], op0=Alu.mult, op1=Alu.mult)
                consts = (cq, sq_, ck, sk)
            return xT, rstdB, consts

        xr = rms_pass(hT_d, 0, "a", BF16, True)
        for sh in range(2):
            x1T, rstdB, (cq, sq_, ck, sk) = xr
            qTa = qk2.tile([128, NHC * SH], F16, tag="qTa")
            # ---- qkv for all heads, this half ----
            for ot in range(12):
                wsl = wqp.tile([128, NKT * 128], BF16, tag="wsl", name="wsl")
                nc.sync.dma_start(out=wsl[:],
                                  in_=wqkv_d[ot * 128:(ot + 1) * 128, :])
                mm = [pp.tile([128, 512], F32, tag="mm", name="mm")
                      for _ in range(2)]
                for kt in range(NKT):
                    for c2 in range(2):
                        nc.tensor.matmul(
                            mm[c2][:], wsl[:, kt * 128:(kt + 1) * 128],
                            x1T[:, kt * SH + c2 * 512: kt * SH + c2 * 512 + 512],
                            start=(kt == 0), stop=(kt == NKT - 1),
                            skip_group_check=True)
                h = ot % NHC
                which = ("q", "k", "v")[ot // NHC]
                for ch2 in range(2):
                    cc = ch2 * 512
                    gc0 = sh * SH + cc      # global col
                    if which in ("q", "k"):
                        qsb = f16t()
                        nc.scalar.activation(qsb[:], mm[ch2][:], Act.Copy)
                        qsh = f16t()
                        nc.gpsimd.dma_start(out=qsh[0:64, :],
                                            in_=qsb[64:128, :])
                        nc.gpsimd.dma_start(out=qsh[64:128, :],
                                            in_=qsb[0:64, :])
                        cR, sR = (cq, sq_) if which == "q" else (ck, sk)
                        t1 = f16t()
                        nc.vector.tensor_tensor(
                            out=t1[:], in0=qsb[:],
                            in1=cR[:, cc:cc + 512], op=Alu.mult)
                        t2 = f16t()
                        nc.vector.tensor_tensor(
                            out=t2[:], in0=qsh[:],
                            in1=sR[:, cc:cc + 512], op=Alu.mult)
                        if which == "q":
                            dsl = qTa[:, h * SH + cc: h * SH + cc + 512]
                        else:
                            dsl = kTa[:, h * S + gc0: h * S + gc0 + 512]
                        nc.vector.tensor_tensor(out=dsl, in0=t1[:],
                                                in1=t2[:], op=Alu.add)
                    else:
                        vt = f16t()
                        nc.vector.tensor_tensor(out=vt[:], in0=mm[ch2][:],
                                                in1=rstdB[:, cc:cc + 512],
                                                op=Alu.mult)
                        for st4 in range(4):
                            st = sh * 8 + ch2 * 4 + st4
                            vo = h * NST * 132 + st * 132
                            trp = ptr.tile([128, 128], F16, tag="tr")
                            nc.tensor.transpose(
                                trp[:], vt[:, st4 * 128:(st4 + 1) * 128],
                                id16[:])
                            nc.scalar.activation(
                                Vn[:, vo: vo + 128], trp[:], Act.Copy)
                            nc.vector.memset(Vn[:, vo + 128: vo + 129], 1.0)
            if sh == 0:
                xr = rms_pass(hT_d, SH, "a", BF16, True)
            # ---- attention, q-chunks of this half ----
            oTh = op_.tile([128, NHC * SH], F16, tag="oTh")
            for qs2 in range(2):
                qs = sh * 2 + qs2
                npairs = 4 * qs + 4
                for h in range(NHC):
                    oap = [pro.tile([128, 264], F32, tag="oa", name="oa")
                           for _ in range(2)]
                    for jj in range(npairs):
                        scp = pp.tile([128, 512], F32, tag="mm")
                        nc.tensor.matmul(
                            scp[:], kTa[:, h * S + jj * 128: h * S + (jj + 1) * 128],
                            qTa[:, h * SH + qs2 * 512: h * SH + qs2 * 512 + 512],
                            start=True, stop=True)
                        ET = we.tile([128, 512], BF16, tag="ET",
                                     name="ET")
                        if jj >= 4 * qs:
                            dd = jj - 4 * qs
                            ms = f32t()
                            nc.vector.tensor_tensor(
                                out=ms[:], in0=scp[:],
                                in1=maskt[:, 384 - 128 * dd: 896 - 128 * dd],
                                op=Alu.add)
                            nc.scalar.activation(ET[:], ms[:], Act.Exp)
                        else:
                            nc.scalar.activation(ET[:], scp[:], Act.Exp)
                        for qt in range(4):
                            vo = h * NST * 132 + jj * 132
                            qo = (qt % 2) * 132
                            nc.tensor.matmul(
                                oap[qt // 2][:, qo: qo + 129],
                                ET[:, qt * 128:(qt + 1) * 128],
                                Vn[:, vo: vo + 129],
                                start=(jj == 0 and qt % 2 == 0),
                                stop=(jj == npairs - 1),
                                skip_group_check=True)
                    for qt in range(4):
                        oa = oap[qt // 2][:, (qt % 2) * 132:(qt % 2) * 132 + 132]
                        rz = w2.tile([128, 1], F32, tag="rz")
                        nc.vector.reciprocal(rz[:], oa[:, 128:129])
                        onat = w2.tile([128, 128], F16, tag="onat")
                        nc.vector.tensor_scalar(out=onat[:],
                                                in0=oa[:, 0:128],
                                                scalar1=rz[:], scalar2=None,
                                                op0=Alu.mult)
                        trp = ptr.tile([128, 128], F16, tag="tr")
                        nc.tensor.transpose(trp[:], onat[:], id16[:])
                        so = h * SH + (qs2 * 4 + qt) * 128
                        nc.scalar.activation(oTh[:, so: so + 128], trp[:],
                                             Act.Copy)
            # ---- o-projection partials (+h/8) for this half -> AllReduce ----
            for ot4 in range(0, NKT, 4):
                wsl = wop.tile([128, 4 * NHC * 128], BF16, tag="wos",
                               name="wos")
                nc.sync.dma_start(
                    out=wsl[:],
                    in_=wo_d[(ot4 // 4) * 128:(ot4 // 4 + 1) * 128, :])
                for o4 in range(4):
                    ot = ot4 + o4
                    hsl = hop.tile([128, 1024], BF16, tag="hsl", name="hsl")
                    nc.gpsimd.dma_start(
                        out=hsl[:],
                        in_=hT_d[ot * 128:(ot + 1) * 128,
                                 sh * SH:(sh + 1) * SH])
                    mm = [pp.tile([128, 512], F32, tag="mm", name="mm")
                          for _ in range(2)]
                    for kt in range(NHC):
                        for c2 in range(2):
                            nc.tensor.matmul(
                                mm[c2][:],
                                wsl[:, (o4 * NHC + kt) * 128:
                                    (o4 * NHC + kt + 1) * 128],
                                oTh[:, kt * SH + c2 * 512:
                                    kt * SH + c2 * 512 + 512],
                                start=(kt == 0), stop=(kt == NHC - 1),
                                skip_group_check=True)
                    pt = w8.tile([128, 1024], F16, tag="w8", name="w8")
                    for c2 in range(2):
                        nc.vector.scalar_tensor_tensor(
                            out=pt[:, c2 * 512:c2 * 512 + 512],
                            in0=hsl[:, c2 * 512:c2 * 512 + 512],
                            scalar=1.0 / NCORES, in1=mm[c2][:],
                            op0=Alu.mult, op1=Alu.add)
                    nc.scalar.dma_start(
                        out=part1_d[sh][ot * 128:(ot + 1) * 128, :],
                        in_=pt[:])
            nc.gpsimd.collective_compute(
                "AllReduce", Alu.add,
                replica_groups=[list(range(NCORES))],
                ins=[part1_d[sh][:].opt()], outs=[ar1_d[sh][:].opt()])
        close_pool("pst")
        close_pool("pso")

        # ====================== MLP super-phase =============================
        # Two passes over seq-halves; ar1[p] (= h2) is the input.
        pgu = open_pool(name="psgu", bufs=4, space="PSUM")

        for ps in range(2):
            arck = ar1_d[ps]
            # ---- rms2 over ar (=h2); x2 = ar * rstd (ln2 folded in W) ----
            x2Tc, rstd2B, _ = rms_pass(arck, 0, "b", F16, False)
            for kt in range(NKT):
                xsl = x2Tc[:, kt * SH: kt * SH + SH]
                nc.vector.tensor_tensor(out=xsl, in0=xsl, in1=rstd2B[:],
                                        op=Alu.mult)
            yTc = qk1.tile([128, NIT * SH], F16, tag="Vn", name="yTc")
            # ---- gate/up -> yTc ----
            for it in range(NIT):
                wg = wqp.tile([128, NKT * 128], BF16, tag="wsl", name="wsl")
                nc.sync.dma_start(out=wg[:],
                                  in_=wgu_d[it * 128:(it + 1) * 128, :])
                wu = wqp.tile([128, NKT * 128], BF16, tag="wsl", name="wsl")
                nc.sync.dma_start(
                    out=wu[:],
                    in_=wgu_d[(NIT + it) * 128:(NIT + it + 1) * 128, :])
                gp = [pgu.tile([128, 512], F32, tag="gup", name="gup")
                      for _ in range(2)]
                up = [pgu.tile([128, 512], F32, tag="gup", name="gup")
                      for _ in range(2)]
                for kt in range(NKT):
                    for c2 in range(2):
                        nc.tensor.matmul(
                            gp[c2][:], wg[:, kt * 128:(kt + 1) * 128],
                            x2Tc[:, kt * SH + c2 * 512:
                                 kt * SH + c2 * 512 + 512],
                            start=(kt == 0), stop=(kt == NKT - 1),
                            skip_group_check=True)
                for kt in range(NKT):
                    for c2 in range(2):
                        nc.tensor.matmul(
                            up[c2][:], wu[:, kt * 128:(kt + 1) * 128],
                            x2Tc[:, kt * SH + c2 * 512:
                                 kt * SH + c2 * 512 + 512],
                            start=(kt == 0), stop=(kt == NKT - 1),
                            skip_group_check=True)
                for c2 in range(2):
                    sg = f16t()
                    nc.scalar.activation(sg[:], gp[c2][:], Act.Silu)
                    nc.vector.tensor_tensor(
                        out=yTc[:, it * SH + c2 * 512:
                                it * SH + c2 * 512 + 512],
                        in0=sg[:], in1=up[c2][:], op=Alu.mult)
            # ---- down partials (+h2/8), per 512-chunk -> ReduceScatter ----
            for ch2 in range(2):
                gch = ps * 2 + ch2          # global 512-col chunk
                cc = ch2 * 512
                for ot2 in range(0, NKT, 2):
                    wsl = wop.tile([128, 2 * NIT * 128], BF16, tag="wos",
                                   name="wos")
                    nc.sync.dma_start(
                        out=wsl[:],
                        in_=wdn_d[(ot2 // 2) * 128:(ot2 // 2 + 1) * 128, :])
                    hsl = hop.tile([128, 1024], F16, tag="hsl", name="hsl")
                    nc.gpsimd.dma_start(
                        out=hsl[:].rearrange("p (o s) -> p o s", s=512),
                        in_=arck[ot2 * 128:(ot2 + 2) * 128, cc:cc + 512]
                        .rearrange("(o p) s -> p o s", p=128))
                    for o2 in range(2):
                        ot = ot2 + o2
                        mm = pp.tile([128, 512], F32, tag="mm")
                        for kt in range(NIT):
                            nc.tensor.matmul(
                                mm[:],
                                wsl[:, (o2 * NIT + kt) * 128:
                                    (o2 * NIT + kt + 1) * 128],
                                yTc[:, kt * SH + cc: kt * SH + cc + 512],
                                start=(kt == 0), stop=(kt == NIT - 1))
                        pt = f16t()
                        nc.vector.scalar_tensor_tensor(
                            out=pt[:], in0=hsl[:, o2 * 512:o2 * 512 + 512],
                            scalar=1.0 / NCORES, in1=mm[:],
                            op0=Alu.mult, op1=Alu.add)
                        nc.scalar.dma_start(
                            out=part2_d[gch][ot * 128:(ot + 1) * 128, :],
                            in_=pt[:])
                nc.gpsimd.collective_compute(
                    "ReduceScatter", Alu.add,
                    replica_groups=[list(range(NCORES))],
                    ins=[part2_d[gch][:].opt()], outs=[rs2_d[gch][:].opt()])
                nc.sync.dma_start(out=out_d[:, gch * 512:(gch + 1) * 512],
                                  in_=rs2_d[gch][:])
        close_pool("psgu")

        for cm, nm in reversed(ctx_pools):
            cm.__exit__(None, None, None)
        ctx_pools.clear()

    nc.compile()
    return nc


def _dequant_np(qw, qz, sc):
    """GPTQ int4 -> float32 weight matrix [in_f, out_f] (numpy)."""
    in_f, out_f = qw.shape[0] * 8, qw.shape[1]
    groups = sc.shape[0]
    sh = (np.arange(8, dtype=np.uint32) * 4)
    w = ((qw.view(np.uint32)[:, None, :] >> sh[None, :, None]) & 15)
    w = w.reshape(groups, in_f // groups, out_f).astype(np.float32)
    z = ((qz.view(np.uint32)[:, :, None] >> sh[None, None, :]) & 15)
    z = z.reshape(groups, out_f).astype(np.float32)
    return ((w - (z + 1.0)[:, None, :]) * sc[:, None, :]).reshape(in_f, out_f)


def _slab(w, nkt, batch=1):
    """[in_f, out_f] f32 -> slabs [(notile//batch)*128, batch*nkt*128] bf16.

    Row block b holds, for each k-partition p, the [ot-in-batch][kt][o]
    weights — so one plain 2D DMA delivers a ready lhsT slab."""
    import ml_dtypes
    in_f, out_f = w.shape
    assert in_f == nkt * 128
    notile = out_f // 128
    s = w.reshape(nkt, 128, notile, 128).transpose(2, 0, 1, 3)  # [ot,kt,p,o]
    s = s.reshape(notile // batch, batch, nkt, 128, 128)
    s = s.transpose(0, 3, 1, 2, 4)                        # [b, p, bot, kt, o]
    return np.ascontiguousarray(
        s.reshape(notile // batch * 128,
                  batch * nkt * 128)).astype(ml_dtypes.bfloat16)


def _host_prep(inputs):
    """Build the 8 per-core input maps from full inputs."""
    import ml_dtypes
    bf16 = ml_dtypes.bfloat16
    f16 = np.float16

    h = np.asarray(inputs["hidden_states"], np.float32)[0]     # [S, HID]
    hT = np.ascontiguousarray(h.T).astype(bf16)                # [HID, S]
    sin = np.asarray(inputs["sin"], np.float32)                # [S, HD]
    cos = np.asarray(inputs["cos"], np.float32)
    cosT = np.ascontiguousarray(cos.T).astype(f16)
    sinf = sin.T.copy()
    sinf[0:64, :] *= -1.0                                      # rot-half sign fold
    sinfT = np.ascontiguousarray(sinf).astype(f16)

    dq = {}
    for nm in ("q", "k", "v", "o", "gate", "up", "down"):
        dq[nm] = _dequant_np(np.asarray(inputs["qw_" + nm]),
                             np.asarray(inputs["qz_" + nm]),
                             np.asarray(inputs["sc_" + nm], np.float32))

    ln1 = np.asarray(inputs["ln1_w"], np.float32)
    ln2 = np.asarray(inputs["ln2_w"], np.float32)
    for nm in ("q", "k", "v"):
        dq[nm] = dq[nm] * ln1[:, None]
    dq["gate"] = dq["gate"] * ln2[:, None]
    dq["up"] = dq["up"] * ln2[:, None]

    def pad_cols(a, wdt):
        out = np.zeros((a.shape[0], wdt), a.dtype)
        out[:, :a.shape[1]] = a
        return out

    w_g = pad_cols(dq["gate"], IPAD)
    w_u = pad_cols(dq["up"], IPAD)
    w_dn = np.zeros((IPAD, HID), np.float32)
    w_dn[:INTER] = dq["down"]

    maps = []
    for c in range(NCORES):
        cs = slice(c * OPC, (c + 1) * OPC)
        isl = slice(c * IPC, (c + 1) * IPC)
        qkv_cat = np.concatenate(
            [dq["q"][:, cs], dq["k"][:, cs], dq["v"][:, cs]], axis=1)
        gu_cat = np.concatenate([w_g[:, isl], w_u[:, isl]], axis=1)
        m = {
            "hT": hT, "cosT": cosT, "sinfT": sinfT,
            "wqkv": _slab(qkv_cat, NKT),
            "wo": _slab(dq["o"][cs, :], NHC, batch=4),
            "wgu": _slab(gu_cat, NKT),
            "wdn": _slab(w_dn[isl, :], NIT, batch=2),
        }
        maps.append(m)
    return maps


def run(inputs, trace=False):
    from concourse.bass_utils import run_bass_kernel_spmd
    if "rel" not in _BUILD_CACHE:
        _BUILD_CACHE["rel"] = _build()
    nc = _BUILD_CACHE["rel"]
    maps = _host_prep(inputs)
    res = run_bass_kernel_spmd(nc, maps, core_ids=list(range(NCORES)),
                               trace=trace)
    outT = np.empty((HID, S), np.float32)
    for c in range(NCORES):
        outT[c * OPC:(c + 1) * OPC] = res.results[c]["out"]
    out = np.ascontiguousarray(outT.T)[None]
    return out, res


def kernel(**inputs):
    out, _ = run(inputs)
    return out


# revision 37
# speedup vs baseline: 1.0245x; 1.0146x over previous
"""ExLlama transformer layer (GPTQ int4) on 8 TRN2 NeuronCores, tensor-parallel.

Self-contained: hardcodes shapes from the problem spec.
  B=1, S=2048, HID=4096, INTER=11008, HEADS=32, HD=128, GS=128.

v3: host-side dequantization to bf16 with ln1/ln2 folded into the qkv /
gate/up weights; the rms normalization factor (rstd) is applied POST-matmul
(folded into the rope cos/sin tables per column and the v/x scaling), so
qkv matmuls depend only on the raw h stash.  h/ar strips DMA directly into
the stash layout (no copy).  o partials carry +h/8 so the AllReduce output
is h2.  MLP runs in 2 half-sequence passes; pools/tags are reused across
phases so SBUF addresses hand over without phase-boundary stalls.  The
final ReduceScatter runs per 512-column chunk, overlapped under compute.
DMA issue is spread across the sync/scalar/gpsimd queues.
"""
import sys

sys.path.insert(0, "/opt/trn_rl_repo")

import numpy as np

S = 2048
HID = 4096
HD = 128
GS = 128
INTER = 11008
NCORES = 8
IPC = 1408                      # padded inter features per core
IPAD = IPC * NCORES             # 11264
NKT = HID // 128                # 32 k-tiles over HID
NIT = IPC // 128                # 11 k-tiles over per-core inter
OPC = HID // NCORES             # 512 out features per core (qkv), 4 heads
NHC = OPC // HD                 # 4 heads per core
NST = S // 128                  # 16 s-tiles
SH = S // 2                     # 1024, seq half
SCALE = 1.0 / float(np.sqrt(HD))
EPS = 1e-6
NEG = -30000.0

_BUILD_CACHE = {}


def _build():
    import concourse.bacc as bacc
    import concourse.mybir as mybir
    import concourse.tile as tile
    import ml_dtypes

    dt = mybir.dt
    F32, F16, BF16, I32 = dt.float32, dt.float16, dt.bfloat16, dt.int32
    Alu = mybir.AluOpType
    Act = mybir.ActivationFunctionType

    nc = bacc.Bacc("TRN2", target_bir_lowering=False, num_devices=NCORES)

    # ---------------- external I/O ----------------
    hT_d = nc.dram_tensor("hT", [HID, S], BF16, kind="ExternalInput")
    cosT_d = nc.dram_tensor("cosT", [HD, S], F16, kind="ExternalInput")
    sinfT_d = nc.dram_tensor("sinfT", [HD, S], F16, kind="ExternalInput")
    # bf16 weight slabs pre-arranged so each slab DMA is a plain 2D copy:
    # row block = [128 k-partitions, batch*nkt*128] per (batched) out-tile
    wqkv_d = nc.dram_tensor("wqkv", [12 * 128, NKT * 128], BF16,
                            kind="ExternalInput")
    wo_d = nc.dram_tensor("wo", [8 * 128, 4 * NHC * 128], BF16,
                          kind="ExternalInput")
    wgu_d = nc.dram_tensor("wgu", [2 * NIT * 128, NKT * 128], BF16,
                           kind="ExternalInput")
    wdn_d = nc.dram_tensor("wdn", [16 * 128, 2 * NIT * 128], BF16,
                           kind="ExternalInput")

    out_d = nc.dram_tensor("out", [OPC, S], F16, kind="ExternalOutput")

    # ---------------- inline constants ----------------
    p = np.arange(128)
    id16_c = nc.inline_tensor(np.eye(128, dtype=np.float16), name="id16")
    ones16_c = nc.inline_tensor(np.ones((128, 1), np.float16), name="ones16")
    # wide causal mask, S^T layout: mkw[k, j] with qq = j-384: 0 if qq >= k
    # else NEG.  Slice [384-128*dd : 896-128*dd] = mask for diag offset dd.
    j = np.arange(896)
    mkw = np.where((j[None, :] - 384) >= p[:, None], 0.0, NEG)
    mask_c = nc.inline_tensor(mkw.astype(ml_dtypes.bfloat16), name="maskc")

    with tile.TileContext(nc) as tc:
        ctx_pools = []

        def open_pool(**kw):
            cm = tc.tile_pool(**kw)
            pool = cm.__enter__()
            ctx_pools.append((cm, kw["name"]))
            return pool

        def close_pool(pool_name):
            for i, (cm, nm) in enumerate(ctx_pools):
                if nm == pool_name:
                    cm.__exit__(None, None, None)
                    ctx_pools.pop(i)
                    return

        cp = open_pool(name="const", bufs=1)
        dp = open_pool(name="dram", bufs=1, space="DRAM")
        w4 = open_pool(name="wk4", bufs=2)    # f32t [128,512]
        w6 = open_pool(name="wk6", bufs=3)    # f16t [128,512]
        w8 = open_pool(name="wk8", bufs=2)    # wide f16 [128,1024]
        we = open_pool(name="wke", bufs=3)    # ET [128,512] bf16
        w3 = open_pool(name="wk3", bufs=1)    # rows [1,*]
        w2 = open_pool(name="wk2", bufs=2)    # onat, rz
        rp = open_pool(name="rope", bufs=1)   # per-half rope consts (wide)
        pp = open_pool(name="ps", bufs=3, space="PSUM")
        prs = open_pool(name="psr", bufs=1, space="PSUM")

        # ---- persistent consts in SBUF ----
        id16 = cp.tile([128, 128], F16, tag="id16")
        nc.sync.dma_start(out=id16[:], in_=id16_c[:])
        ones16 = cp.tile([128, 1], F16, tag="ones16")
        nc.sync.dma_start(out=ones16[:], in_=ones16_c[:])
        maskt = cp.tile([128, 896], BF16, tag="maskt")
        nc.sync.dma_start(out=maskt[:], in_=mask_c[:])


        # ---- DRAM scratch ----
        part1_d = [dp.tile([HID, SH], F16, tag=f"part1_{i}", name=f"part1_{i}")
                   for i in range(2)]
        ar1_d = [dp.tile([HID, SH], F16, tag=f"ar1_{i}", name=f"ar1_{i}",
                         addr_space="Shared")
                 for i in range(2)]
        part2_d = [dp.tile([HID, 512], F16, tag=f"part2_{i}",
                           name=f"part2_{i}") for i in range(4)]
        rs2_d = [dp.tile([OPC, 512], F16, tag=f"rs2_{i}", name=f"rs2_{i}")
                 for i in range(4)]

        def f32t():
            return w4.tile([128, 512], F32, tag="f32t", name="f32t")

        def f16t(dtp=F16):
            return w6.tile([128, 512], dtp, tag="f16t", name="f16t")

        # =============== SBUF pools reused across phases ===============
        xp = open_pool(name="xph", bufs=1)    # x1T / x2Tc (same tag slot)
        qk2 = open_pool(name="qk2", bufs=1)   # qTa
        qk1 = open_pool(name="qk1", bufs=1)   # kTa, Vn (later yTc)
        op_ = open_pool(name="oTp", bufs=2)   # oTh
        wqp = open_pool(name="wq", bufs=3)    # [128,4096] slabs: qkv, gate/up
        wop = open_pool(name="wo", bufs=2)    # o (2048) / down (2816) slabs
        hop = open_pool(name="ho", bufs=2)    # h / h2 row slices
        pro = open_pool(name="pso", bufs=3, space="PSUM")
        ptr = open_pool(name="pst", bufs=1, space="PSUM")

        kTa = qk1.tile([128, NHC * S], F16, tag="kTa")
        Vn = qk1.tile([128, NHC * NST * 132], BF16, tag="Vn")

        def rms_pass(src_d, c0, stash_tag, stash_dt, with_rope):
            """DMA 1024 cols of src into stash layout, ssq -> rstd; build
            per-half rope consts (rstd folded).  Returns (stash, consts)."""
            xT = xp.tile([128, NKT * SH], stash_dt, tag="x1T", name="x1T")
            ssq = prs.tile([64, 512], F32, tag="rs")
            for st in range(8):
                nc.gpsimd.dma_start(
                    out=xT[:, st * 4 * SH:(st * 4 + 4) * SH]
                    .rearrange("p (kt s) -> p kt s", s=SH),
                    in_=src_d[st * 512:(st + 1) * 512, c0:c0 + SH]
                    .rearrange("(kt p) s -> p kt s", p=128))
                for k2 in range(4):
                    kt = st * 4 + k2
                    xsl = xT[:, kt * SH: kt * SH + SH]
                    sq = w8.tile([128, 1024], F16, tag="w8", name="w8")
                    nc.vector.tensor_tensor(out=sq[:], in0=xsl, in1=xsl,
                                            op=Alu.mult)
                    nc.tensor.matmul(ssq[0:1, :], ones16[:], sq[:, 0:512],
                                     start=(kt == 0), stop=(kt == NKT - 1),
                                     skip_group_check=True)
                    nc.tensor.matmul(ssq[32:33, :], ones16[:], sq[:, 512:1024],
                                     start=(kt == 0), stop=(kt == NKT - 1),
                                     skip_group_check=True)
            srw = w3.tile([1, 1024], F16, tag="srw", name="srw")
            for ch2 in range(2):
                trow = w3.tile([1, 512], F32, tag="rows")
                nc.vector.tensor_scalar(out=trow[:],
                                        in0=ssq[32 * ch2:32 * ch2 + 1, :],
                                        scalar1=1.0 / HID, scalar2=EPS,
                                        op0=Alu.mult, op1=Alu.add)
                nc.vector.reciprocal(trow[:], trow[:])
                nc.scalar.activation(srw[0:1, ch2 * 512:ch2 * 512 + 512],
                                     trow[:], Act.Sqrt)
            rstdB = rp.tile([128, 1024], F16, tag="rstdB", name="rstdB")
            nc.gpsimd.partition_broadcast(rstdB[:], srw[:])
            consts = None
            if with_rope:
                csl = rp.tile([128, 1024], F16, tag="csl", name="csl")
                nc.sync.dma_start(out=csl[:], in_=cosT_d[:, c0:c0 + SH])
                ssl = rp.tile([128, 1024], F16, tag="ssl", name="ssl")
                nc.sync.dma_start(out=ssl[:], in_=sinfT_d[:, c0:c0 + SH])
                cq = rp.tile([128, 1024], F16, tag="cq", name="cq")
                nc.vector.tensor_tensor(out=cq[:], in0=csl[:],
                                        in1=rstdB[:], op=Alu.mult)
                sq_ = rp.tile([128, 1024], F16, tag="sq_", name="sq_")
                nc.vector.tensor_tensor(out=sq_[:], in0=ssl[:],
                                        in1=rstdB[:], op=Alu.mult)
                ck = rp.tile([128, 1024], F16, tag="ck", name="ck")
                nc.vector.scalar_tensor_tensor(
                    out=ck[:], in0=csl[:], scalar=SCALE,
                    in1=rstdB[:], op0=Alu.mult, op1=Alu.mult)
                sk = rp.tile([128, 1024], F16, tag="sk", name="sk")
                nc.vector.scalar_tensor_tensor(
                    out=sk[:], in0=ssl[:], scalar=SCALE,
                    in1=rstdB[:], op0=Alu.mult, op1=Alu.mult)
                consts = (cq, sq_, ck, sk)
            return xT, rstdB, consts

        xr = rms_pass(hT_d, 0, "a", BF16, True)
        for sh in range(2):
            x1T, rstdB, (cq, sq_, ck, sk) = xr
            qTa = qk2.tile([128, NHC * SH], F16, tag="qTa")
            # ---- qkv for all heads, this half ----
            for ot in range(12):
                wsl = wqp.tile([128, NKT * 128], BF16, tag="wsl", name="wsl")
                nc.sync.dma_start(out=wsl[:],
                                  in_=wqkv_d[ot * 128:(ot + 1) * 128, :])
                mm = [pp.tile([128, 512], F32, tag="mm", name="mm")
                      for _ in range(2)]
                for kt in range(NKT):
                    for c2 in range(2):
                        nc.tensor.matmul(
                            mm[c2][:], wsl[:, kt * 128:(kt + 1) * 128],
                            x1T[:, kt * SH + c2 * 512: kt * SH + c2 * 512 + 512],
                            start=(kt == 0), stop=(kt == NKT - 1),
                            skip_group_check=True)
                h = ot % NHC
                which = ("q", "k", "v")[ot // NHC]
                for ch2 in range(2):
                    cc = ch2 * 512
                    gc0 = sh * SH + cc      # global col
                    if which in ("q", "k"):
                        qsb = f16t()
                        nc.scalar.activation(qsb[:], mm[ch2][:], Act.Copy)
                        qsh = f16t()
                        nc.gpsimd.dma_start(out=qsh[0:64, :],
                                            in_=qsb[64:128, :])
                        nc.gpsimd.dma_start(out=qsh[64:128, :],
                                            in_=qsb[0:64, :])
                        cR, sR = (cq, sq_) if which == "q" else (ck, sk)
                        t1 = f16t()
                        nc.vector.tensor_tensor(
                            out=t1[:], in0=qsb[:],
                            in1=cR[:, cc:cc + 512], op=Alu.mult)
                        t2 = f16t()
                        nc.vector.tensor_tensor(
                            out=t2[:], in0=qsh[:],
                            in1=sR[:, cc:cc + 512], op=Alu.mult)
                        if which == "q":
                            dsl = qTa[:, h * SH + cc: h * SH + cc + 512]
                        else:
                            dsl = kTa[:, h * S + gc0: h * S + gc0 + 512]
                        nc.vector.tensor_tensor(out=dsl, in0=t1[:],
                                                in1=t2[:], op=Alu.add)
                    else:
                        vt = f16t()
                        nc.vector.tensor_tensor(out=vt[:], in0=mm[ch2][:],
                                                in1=rstdB[:, cc:cc + 512],
                                                op=Alu.mult)
                        for st4 in range(4):
                            st = sh * 8 + ch2 * 4 + st4
                            vo = h * NST * 132 + st * 132
                            trp = ptr.tile([128, 128], F16, tag="tr")
                            nc.tensor.transpose(
                                trp[:], vt[:, st4 * 128:(st4 + 1) * 128],
                                id16[:])
                            nc.scalar.activation(
                                Vn[:, vo: vo + 128], trp[:], Act.Copy)
                            nc.vector.memset(Vn[:, vo + 128: vo + 129], 1.0)
            if sh == 0:
                xr = rms_pass(hT_d, SH, "a", BF16, True)
            # ---- attention, q-chunks of this half ----
            oTh = op_.tile([128, NHC * SH], F16, tag="oTh")
            for qs2 in range(2):
                qs = sh * 2 + qs2
                npairs = 4 * qs + 4
                for h in range(NHC):
                    oap = [pro.tile([128, 264], F32, tag="oa", name="oa")
                           for _ in range(2)]
                    for jj in range(npairs):
                        scp = pp.tile([128, 512], F32, tag="mm")
                        nc.tensor.matmul(
                            scp[:], kTa[:, h * S + jj * 128: h * S + (jj + 1) * 128],
                            qTa[:, h * SH + qs2 * 512: h * SH + qs2 * 512 + 512],
                            start=True, stop=True)
                        ET = we.tile([128, 512], BF16, tag="ET",
                                     name="ET")
                        if jj >= 4 * qs:
                            dd = jj - 4 * qs
                            ms = f32t()
                            nc.vector.tensor_tensor(
                                out=ms[:], in0=scp[:],
                                in1=maskt[:, 384 - 128 * dd: 896 - 128 * dd],
                                op=Alu.add)
                            nc.scalar.activation(ET[:], ms[:], Act.Exp)
                        else:
                            nc.scalar.activation(ET[:], scp[:], Act.Exp)
                        for qt in range(4):
                            vo = h * NST * 132 + jj * 132
                            qo = (qt % 2) * 132
                            nc.tensor.matmul(
                                oap[qt // 2][:, qo: qo + 129],
                                ET[:, qt * 128:(qt + 1) * 128],
                                Vn[:, vo: vo + 129],
                                start=(jj == 0 and qt % 2 == 0),
                                stop=(jj == npairs - 1),
                                skip_group_check=True)
                    for qt in range(4):
                        oa = oap[qt // 2][:, (qt % 2) * 132:(qt % 2) * 132 + 132]
                        rz = w2.tile([128, 1], F32, tag="rz")
                        nc.vector.reciprocal(rz[:], oa[:, 128:129])
                        onat = w2.tile([128, 128], F16, tag="onat")
                        nc.vector.tensor_scalar(out=onat[:],
                                                in0=oa[:, 0:128],
                                                scalar1=rz[:], scalar2=None,
                                                op0=Alu.mult)
                        trp = ptr.tile([128, 128], F16, tag="tr")
                        nc.tensor.transpose(trp[:], onat[:], id16[:])
                        so = h * SH + (qs2 * 4 + qt) * 128
                        nc.scalar.activation(oTh[:, so: so + 128], trp[:],
                                             Act.Copy)
            # ---- o-projection partials (+h/8) for this half -> AllReduce ----
            for ot4 in range(0, NKT, 4):
                wsl = wop.tile([128, 4 * NHC * 128], BF16, tag="wos",
                               name="wos")
                nc.sync.dma_start(
                    out=wsl[:],
                    in_=wo_d[(ot4 // 4) * 128:(ot4 // 4 + 1) * 128, :])
                for o4 in range(4):
                    ot = ot4 + o4
                    hsl = hop.tile([128, 1024], BF16, tag="hsl", name="hsl")
                    nc.gpsimd.dma_start(
                        out=hsl[:],
                        in_=hT_d[ot * 128:(ot + 1) * 128,
                                 sh * SH:(sh + 1) * SH])
                    mm = [pp.tile([128, 512], F32, tag="mm", name="mm")
                          for _ in range(2)]
                    for kt in range(NHC):
                        for c2 in range(2):
                            nc.tensor.matmul(
                                mm[c2][:],
                                wsl[:, (o4 * NHC + kt) * 128:
                                    (o4 * NHC + kt + 1) * 128],
                                oTh[:, kt * SH + c2 * 512:
                                    kt * SH + c2 * 512 + 512],
                                start=(kt == 0), stop=(kt == NHC - 1),
                                skip_group_check=True)
                    pt = w8.tile([128, 1024], F16, tag="w8", name="w8")
                    for c2 in range(2):
                        nc.vector.scalar_tensor_tensor(
                            out=pt[:, c2 * 512:c2 * 512 + 512],
                            in0=hsl[:, c2 * 512:c2 * 512 + 512],
                            scalar=1.0 / NCORES, in1=mm[c2][:],
                            op0=Alu.mult, op1=Alu.add)
                    nc.scalar.dma_start(
                        out=part1_d[sh][ot * 128:(ot + 1) * 128, :],
                        in_=pt[:])
            nc.gpsimd.collective_compute(
                "AllReduce", Alu.add,
                replica_groups=[list(range(NCORES))],
                ins=[part1_d[sh][:].opt()], outs=[ar1_d[sh][:].opt()])
        close_pool("pst")
        close_pool("pso")

        # ====================== MLP super-phase =============================
        # Two passes over seq-halves; ar1[p] (= h2) is the input.
        pgu = open_pool(name="psgu", bufs=4, space="PSUM")

        def rms2_block(ps):
            # ---- rms2 over ar (=h2); x2 = ar * rstd (ln2 folded in W) ----
            x2Tc, rstd2B, _ = rms_pass(ar1_d[ps], 0, "b", F16, False)
            for kt in range(NKT):
                xsl = x2Tc[:, kt * SH: kt * SH + SH]
                nc.vector.tensor_tensor(out=xsl, in0=xsl, in1=rstd2B[:],
                                        op=Alu.mult)
            return x2Tc

        x2r = rms2_block(0)
        for ps in range(2):
            arck = ar1_d[ps]
            x2Tc = x2r
            yTc = qk1.tile([128, NIT * SH], F16, tag="Vn", name="yTc")
            # ---- gate/up -> yTc ----
            for it in range(NIT):
                wg = wqp.tile([128, NKT * 128], BF16, tag="wsl", name="wsl")
                nc.sync.dma_start(out=wg[:],
                                  in_=wgu_d[it * 128:(it + 1) * 128, :])
                wu = wqp.tile([128, NKT * 128], BF16, tag="wsl", name="wsl")
                nc.sync.dma_start(
                    out=wu[:],
                    in_=wgu_d[(NIT + it) * 128:(NIT + it + 1) * 128, :])
                gp = [pgu.tile([128, 512], F32, tag="gup", name="gup")
                      for _ in range(2)]
                up = [pgu.tile([128, 512], F32, tag="gup", name="gup")
                      for _ in range(2)]
                for kt in range(NKT):
                    for c2 in range(2):
                        nc.tensor.matmul(
                            gp[c2][:], wg[:, kt * 128:(kt + 1) * 128],
                            x2Tc[:, kt * SH + c2 * 512:
                                 kt * SH + c2 * 512 + 512],
                            start=(kt == 0), stop=(kt == NKT - 1),
                            skip_group_check=True)
                for kt in range(NKT):
                    for c2 in range(2):
                        nc.tensor.matmul(
                            up[c2][:], wu[:, kt * 128:(kt + 1) * 128],
                            x2Tc[:, kt * SH + c2 * 512:
                                 kt * SH + c2 * 512 + 512],
                            start=(kt == 0), stop=(kt == NKT - 1),
                            skip_group_check=True)
                for c2 in range(2):
                    sg = f16t()
                    nc.scalar.activation(sg[:], gp[c2][:], Act.Silu)
                    nc.vector.tensor_tensor(
                        out=yTc[:, it * SH + c2 * 512:
                                it * SH + c2 * 512 + 512],
                        in0=sg[:], in1=up[c2][:], op=Alu.mult)
            # ---- down partials (+h2/8), per 512-chunk -> ReduceScatter ----
            for ch2 in range(2):
                gch = ps * 2 + ch2          # global 512-col chunk
                cc = ch2 * 512
                for ot2 in range(0, NKT, 2):
                    wsl = wop.tile([128, 2 * NIT * 128], BF16, tag="wos",
                                   name="wos")
                    nc.sync.dma_start(
                        out=wsl[:],
                        in_=wdn_d[(ot2 // 2) * 128:(ot2 // 2 + 1) * 128, :])
                    hsl = hop.tile([128, 1024], F16, tag="hsl", name="hsl")
                    nc.gpsimd.dma_start(
                        out=hsl[:].rearrange("p (o s) -> p o s", s=512),
                        in_=arck[ot2 * 128:(ot2 + 2) * 128, cc:cc + 512]
                        .rearrange("(o p) s -> p o s", p=128))
                    for o2 in range(2):
                        ot = ot2 + o2
                        mm = pp.tile([128, 512], F32, tag="mm")
                        for kt in range(NIT):
                            nc.tensor.matmul(
                                mm[:],
                                wsl[:, (o2 * NIT + kt) * 128:
                                    (o2 * NIT + kt + 1) * 128],
                                yTc[:, kt * SH + cc: kt * SH + cc + 512],
                                start=(kt == 0), stop=(kt == NIT - 1))
                        pt = f16t()
                        nc.vector.scalar_tensor_tensor(
                            out=pt[:], in0=hsl[:, o2 * 512:o2 * 512 + 512],
                            scalar=1.0 / NCORES, in1=mm[:],
                            op0=Alu.mult, op1=Alu.add)
                        nc.scalar.dma_start(
                            out=part2_d[gch][ot * 128:(ot + 1) * 128, :],
                            in_=pt[:])
                if ps == 0 and ch2 == 0:
                    x2r = rms2_block(1)
                nc.gpsimd.collective_compute(
                    "ReduceScatter", Alu.add,
                    replica_groups=[list(range(NCORES))],
                    ins=[part2_d[gch][:].opt()], outs=[rs2_d[gch][:].opt()])
                nc.sync.dma_start(out=out_d[:, gch * 512:(gch + 1) * 512],
                                  in_=rs2_d[gch][:])
        close_pool("psgu")

        for cm, nm in reversed(ctx_pools):
            cm.__exit__(None, None, None)
        ctx_pools.clear()

    nc.compile()
    return nc


def _dequant_np(qw, qz, sc):
    """GPTQ int4 -> float32 weight matrix [in_f, out_f] (numpy)."""
    in_f, out_f = qw.shape[0] * 8, qw.shape[1]
    groups = sc.shape[0]
    sh = (np.arange(8, dtype=np.uint32) * 4)
    w = ((qw.view(np.uint32)[:, None, :] >> sh[None, :, None]) & 15)
    w = w.reshape(groups, in_f // groups, out_f).astype(np.float32)
    z = ((qz.view(np.uint32)[:, :, None] >> sh[None, None, :]) & 15)
    z = z.reshape(groups, out_f).astype(np.float32)
    return ((w - (z + 1.0)[:, None, :]) * sc[:, None, :]).reshape(in_f, out_f)


def _slab(w, nkt, batch=1):
    """[in_f, out_f] f32 -> slabs [(notile//batch)*128, batch*nkt*128] bf16.

    Row block b holds, for each k-partition p, the [ot-in-batch][kt][o]
    weights — so one plain 2D DMA delivers a ready lhsT slab."""
    import ml_dtypes
    in_f, out_f = w.shape
    assert in_f == nkt * 128
    notile = out_f // 128
    s = w.reshape(nkt, 128, notile, 128).transpose(2, 0, 1, 3)  # [ot,kt,p,o]
    s = s.reshape(notile // batch, batch, nkt, 128, 128)
    s = s.transpose(0, 3, 1, 2, 4)                        # [b, p, bot, kt, o]
    return np.ascontiguousarray(
        s.reshape(notile // batch * 128,
                  batch * nkt * 128)).astype(ml_dtypes.bfloat16)


def _host_prep(inputs):
    """Build the 8 per-core input maps from full inputs."""
    import ml_dtypes
    bf16 = ml_dtypes.bfloat16
    f16 = np.float16

    h = np.asarray(inputs["hidden_states"], np.float32)[0]     # [S, HID]
    hT = np.ascontiguousarray(h.T).astype(bf16)                # [HID, S]
    sin = np.asarray(inputs["sin"], np.float32)                # [S, HD]
    cos = np.asarray(inputs["cos"], np.float32)
    cosT = np.ascontiguousarray(cos.T).astype(f16)
    sinf = sin.T.copy()
    sinf[0:64, :] *= -1.0                                      # rot-half sign fold
    sinfT = np.ascontiguousarray(sinf).astype(f16)

    dq = {}
    for nm in ("q", "k", "v", "o", "gate", "up", "down"):
        dq[nm] = _dequant_np(np.asarray(inputs["qw_" + nm]),
                             np.asarray(inputs["qz_" + nm]),
                             np.asarray(inputs["sc_" + nm], np.float32))

    ln1 = np.asarray(inputs["ln1_w"], np.float32)
    ln2 = np.asarray(inputs["ln2_w"], np.float32)
    for nm in ("q", "k", "v"):
        dq[nm] = dq[nm] * ln1[:, None]
    dq["gate"] = dq["gate"] * ln2[:, None]
    dq["up"] = dq["up"] * ln2[:, None]

    def pad_cols(a, wdt):
        out = np.zeros((a.shape[0], wdt), a.dtype)
        out[:, :a.shape[1]] = a
        return out

    w_g = pad_cols(dq["gate"], IPAD)
    w_u = pad_cols(dq["up"], IPAD)
    w_dn = np.zeros((IPAD, HID), np.float32)
    w_dn[:INTER] = dq["down"]

    maps = []
    for c in range(NCORES):
        cs = slice(c * OPC, (c + 1) * OPC)
        isl = slice(c * IPC, (c + 1) * IPC)
        qkv_cat = np.concatenate(
            [dq["q"][:, cs], dq["k"][:, cs], dq["v"][:, cs]], axis=1)
        gu_cat = np.concatenate([w_g[:, isl], w_u[:, isl]], axis=1)
        m = {
            "hT": hT, "cosT": cosT, "sinfT": sinfT,
            "wqkv": _slab(qkv_cat, NKT),
            "wo": _slab(dq["o"][cs, :], NHC, batch=4),
            "wgu": _slab(gu_cat, NKT),
            "wdn": _slab(w_dn[isl, :], NIT, batch=2),
        }
        maps.append(m)
    return maps


def run(inputs, trace=False):
    from concourse.bass_utils import run_bass_kernel_spmd
    if "rel" not in _BUILD_CACHE:
        _BUILD_CACHE["rel"] = _build()
    nc = _BUILD_CACHE["rel"]
    maps = _host_prep(inputs)
    res = run_bass_kernel_spmd(nc, maps, core_ids=list(range(NCORES)),
                               trace=trace)
    outT = np.empty((HID, S), np.float32)
    for c in range(NCORES):
        outT[c * OPC:(c + 1) * OPC] = res.results[c]["out"]
    out = np.ascontiguousarray(outT.T)[None]
    return out, res


def kernel(**inputs):
    out, _ = run(inputs)
    return out
